# revision 33
# baseline (speedup 1.0000x reference)
"""Trainium2 kernel for nn_MergeModuleTemporal (retrieval_knn greedy merge).

Strategy:
  - Per merge round, the O(n^2 d) similarity GEMM runs on 8 NeuronCores as a
    split-bf16 3-term matmul (hi*hi + hi*lo + lo*hi) with fp32 PSUM
    accumulation -> ~1e-7 accurate vs exact fp32, which preserves every greedy
    argmax decision of the fp32 reference on this regime (decision margins are
    >= ~2.6e-7 except exact ties between bit-identical rows, which any
    deterministic matmul also ties; ties break by index).
  - The greedy scan only ever reads sim[i, j] for j > i, so each round
    computes only upper-triangular n-tile bands: m-tiles are grouped in bands
    of 8 (one per core, band t covers output rows [1024t, 1024t+1024)), and
    band t's matmuls start at column 1024t. This keeps the program identical
    across cores (SPMD) while skipping ~40% of the GEMM.
  - Rounds 1..3 compact to alive rows only (4096 -> ~2245 -> ~1273 -> ~787),
    shrinking the GEMM quadratically; per-shape programs are cached.
  - The sequential greedy matching (a tiny [n] state machine) and the exact
    fp32 fuse (clamp(a+b, max=1)) run on host between the 4 round launches;
    fp32 adds/min are bit-exact vs the jax reference.
"""

import numpy as np
import ml_dtypes
from contextlib import ExitStack

N_ROWS = 4096
D = 1024
NCORES = 8
P = 128
KO = D // P            # 8 contraction chunks of 128
MAX_STEPS = 4
THR = np.float32(0.25)
NEG = np.float32(-1e9)
NT = 512               # matmul free-dim tile
PSUM_GROUP = 4         # n-tiles sharing one k-sweep (4 tags x 2 bufs = 8 banks)
BAND = NCORES * P      # 1024 output rows per band

_PROGS = {}


def _build_program(n_pad, nbands):
    """One merge-round triangular similarity GEMM. Per core: for each band
    slot t, compute out[t*128:(t+1)*128, 1024*t:] = lhs_t.T @ rhs / D where
    lhs_t is that core's m-tile of band t (cols t*128..t*128+128 of lhs)."""
    import concourse.tile as tile
    import concourse.mybir as mybir
    from concourse import bacc

    assert n_pad % 256 == 0

    nc = bacc.Bacc(
        "TRN2",
        target_bir_lowering=False,
        debug=False,
        enable_asserts=False,
        num_devices=NCORES,
    )
    bf16 = mybir.dt.bfloat16
    f32 = mybir.dt.float32
    mcap = nbands * P  # output rows per core
    lhs_hi = nc.dram_tensor("lhs_hi", [P, KO, mcap], bf16, kind="ExternalInput").ap()
    lhs_lo = nc.dram_tensor("lhs_lo", [P, KO, mcap], bf16, kind="ExternalInput").ap()
    rhs_hi = nc.dram_tensor("rhs_hi", [P, KO, n_pad], bf16, kind="ExternalInput").ap()
    rhs_lo = nc.dram_tensor("rhs_lo", [P, KO, n_pad], bf16, kind="ExternalInput").ap()
    out = nc.dram_tensor("sim", [mcap, n_pad], f32, kind="ExternalOutput").ap()

    with tile.TileContext(nc) as tc:
        with ExitStack() as ctx:
            sb = ctx.enter_context(tc.tile_pool(name="sb", bufs=1))
            psum = ctx.enter_context(tc.tile_pool(name="psum", bufs=2, space="PSUM"))
            outp = ctx.enter_context(tc.tile_pool(name="outp", bufs=4))

            lh = sb.tile([P, KO, mcap], bf16)
            ll = sb.tile([P, KO, mcap], bf16)
            rh = sb.tile([P, KO, n_pad], bf16)
            rl = sb.tile([P, KO, n_pad], bf16)
            # Load order matches consumption order so the first matmuls can
            # start early instead of waiting for the whole operand set.
            # Granularity is shape-tuned: finer splits help big shapes but
            # fragment the DMA queue on small ones (cost-model A/B'd).
            if n_pad >= 4096:
                # Bands run high-to-low (cheapest slot first), so deliver the
                # upper column half first, k-major. k=0's upper half goes in
                # quarters (highest first) so the very first matmul only
                # waits on ~0.25MB.
                nh = n_pad // 2
                nq = (n_pad + nh) // 2
                for k in range(KO):
                    nc.sync.dma_start(lh[:, k], lhs_hi[:, k])
                    if k == 0:
                        nc.sync.dma_start(rh[:, k, nq:], rhs_hi[:, k, nq:])
                        nc.sync.dma_start(rl[:, k, nq:], rhs_lo[:, k, nq:])
                        nc.sync.dma_start(ll[:, k], lhs_lo[:, k])
                        nc.sync.dma_start(rh[:, k, nh:nq], rhs_hi[:, k, nh:nq])
                        nc.sync.dma_start(rl[:, k, nh:nq], rhs_lo[:, k, nh:nq])
                    else:
                        nc.sync.dma_start(ll[:, k], lhs_lo[:, k])
                        nc.sync.dma_start(rh[:, k, nh:], rhs_hi[:, k, nh:])
                        nc.sync.dma_start(rl[:, k, nh:], rhs_lo[:, k, nh:])
                for k in range(KO):
                    nc.sync.dma_start(rh[:, k, :nh], rhs_hi[:, k, :nh])
                    nc.sync.dma_start(rl[:, k, :nh], rhs_lo[:, k, :nh])
            elif n_pad >= 2048:
                for k in range(KO):
                    nc.sync.dma_start(lh[:, k], lhs_hi[:, k])
                    nc.sync.dma_start(ll[:, k], lhs_lo[:, k])
                    nc.sync.dma_start(rh[:, k], rhs_hi[:, k])
                    nc.sync.dma_start(rl[:, k], rhs_lo[:, k])
            else:
                nc.sync.dma_start(lh[:], lhs_hi)
                nc.sync.dma_start(ll[:], lhs_lo)
                for k in range(KO):
                    nc.sync.dma_start(rh[:, k], rhs_hi[:, k])
                    nc.sync.dma_start(rl[:, k], rhs_lo[:, k])

            reverse = n_pad >= 4096
            sched = []
            for t in (reversed(range(nbands)) if reverse else range(nbands)):
                n0 = min(t * BAND, n_pad)  # multiple of 512
                n_tiles = [(ns, min(NT, n_pad - ns)) for ns in range(n0, n_pad, NT)]
                groups = [
                    n_tiles[g0:g0 + PSUM_GROUP]
                    for g0 in range(0, len(n_tiles), PSUM_GROUP)
                ]
                for grp in (reversed(groups) if reverse else groups):
                    sched.append((t, grp))
            # Split the final group into singletons: the last stores are on
            # the critical path, and singleton groups pipeline copy+DMA per
            # tile instead of exposing the whole group's stores at the end.
            if sched and len(sched[-1][1]) > 1:
                t_last, grp_last = sched.pop()
                sched.extend((t_last, [nt]) for nt in grp_last)
            for t, grp in sched:
                ms = t * P
                pss = [
                    psum.tile([P, NT], f32, name=f"ps{j}")
                    for j in range(len(grp))
                ]
                # k-sweep over the group: weights load once per (k, term)
                # and serve len(grp) matmuls.
                for k in range(KO):
                    for ti, (lt, rt) in enumerate(((lh, rh), (lh, rl), (ll, rh))):
                        for j, (ns, nsz) in enumerate(grp):
                            nc.tensor.matmul(
                                pss[j][:, :nsz],
                                lt[:, k, ms:ms + P],
                                rt[:, k, ns:ns + nsz],
                                start=(k == 0 and ti == 0),
                                stop=(k == KO - 1 and ti == 2),
                            )
                for j, (ns, nsz) in enumerate(grp):
                    ob = outp.tile([P, NT], f32, name="ob")
                    nc.scalar.mul(ob[:, :nsz], pss[j][:, :nsz], 1.0 / D)
                    nc.sync.dma_start(out[ms:ms + P, ns:ns + nsz], ob[:, :nsz])

    nc.compile()
    return nc


def _device_sim(emb_c):
    """Upper-triangular sim = emb_c @ emb_c.T / D (fp32-accurate) on 8 cores.
    emb_c: [n_alive, D] fp32. Returns [n_alive, n_alive] fp32 with only
    j >= 1024*(i//1024-band) guaranteed valid -- in particular all j > i."""
    from concourse.bass_utils import run_bass_kernel_spmd

    n_alive = emb_c.shape[0]
    n_pad = max(256, -(-n_alive // 256) * 256)
    ntiles_m = -(-n_alive // P)
    nbands = -(-ntiles_m // NCORES)
    mcap = nbands * P

    key = (n_pad, nbands)
    if key not in _PROGS:
        _PROGS[key] = _build_program(n_pad, nbands)
    prog = _PROGS[key]

    hi = emb_c.astype(ml_dtypes.bfloat16)
    lo = (emb_c - hi.astype(np.float32)).astype(ml_dtypes.bfloat16)
    # [n, d] -> ET layout [P, KO, n_pad]  (ET[p, k, j] = emb[j, k*P + p])
    et_hi = np.zeros((P, KO, n_pad), dtype=ml_dtypes.bfloat16)
    et_lo = np.zeros((P, KO, n_pad), dtype=ml_dtypes.bfloat16)
    et_hi[:, :, :n_alive] = hi.T.reshape(KO, P, n_alive).transpose(1, 0, 2)
    et_lo[:, :, :n_alive] = lo.T.reshape(KO, P, n_alive).transpose(1, 0, 2)

    in_maps = []
    for c in range(NCORES):
        lhs_hi = np.zeros((P, KO, mcap), dtype=ml_dtypes.bfloat16)
        lhs_lo = np.zeros((P, KO, mcap), dtype=ml_dtypes.bfloat16)
        for t in range(nbands):
            mo = NCORES * t + c
            cs = mo * P
            if cs < n_pad:
                ce = min(cs + P, n_pad)
                lhs_hi[:, :, t * P:t * P + (ce - cs)] = et_hi[:, :, cs:ce]
                lhs_lo[:, :, t * P:t * P + (ce - cs)] = et_lo[:, :, cs:ce]
        in_maps.append({
            "lhs_hi": lhs_hi,
            "lhs_lo": lhs_lo,
            "rhs_hi": et_hi,
            "rhs_lo": et_lo,
        })
    res = run_bass_kernel_spmd(prog, in_maps, core_ids=list(range(NCORES)))
    sim = np.zeros((n_pad, n_pad), dtype=np.float32)
    for c in range(NCORES):
        oc = res.results[c]["sim"]  # [mcap, n_pad]
        for t in range(nbands):
            mo = NCORES * t + c
            cs = mo * P
            if cs < n_pad:
                ce = min(cs + P, n_pad)
                sim[cs:ce] = oc[t * P:t * P + (ce - cs)]
    return sim[:n_alive, :n_alive]


def _scan(sim):
    """Exact replication of the reference greedy matching on the compacted
    (all-alive) sim matrix; only reads sim[i, j>i]. argmax ties -> lowest
    index."""
    n = sim.shape[0]
    merged = np.zeros(n, dtype=bool)
    partner = np.full(n, -1)
    idx = np.arange(n)
    for i in range(n):
        cand = (idx > i) & (~merged)
        if not cand.any():
            continue
        scores = np.where(cand, sim[i], NEG)
        j = int(np.argmax(scores))
        if (not merged[i]) and (scores[j] >= THR):
            partner[i] = j
            merged[j] = True
    return partner


def kernel(embeddings):
    emb = np.asarray(embeddings, dtype=np.float32).copy()
    n = emb.shape[0]
    alive = np.ones(n, dtype=bool)
    done = False
    for _ in range(MAX_STEPS):
        if done:
            break
        alive_idx = np.nonzero(alive)[0]
        emb_c = np.ascontiguousarray(emb[alive_idx])
        sim_c = _device_sim(emb_c)
        partner_c = _scan(sim_c)
        has_c = partner_c >= 0
        occurred = bool(has_c.any())
        if occurred:
            # Map compacted indices back and replicate the reference's fp32
            # fuse exactly: fused = min(e_i + e_partner, 1); consumed -> 0.
            partner = np.full(n, -1)
            partner[alive_idx[has_c]] = alive_idx[partner_c[has_c]]
            has = partner >= 0
            consumed = np.zeros(n, dtype=bool)
            consumed[partner[has]] = True
            pidx = np.clip(partner, 0, n - 1)
            fused = np.minimum(emb + emb[pidx], np.float32(1.0))
            emb = np.where(has[:, None], fused, emb)
            emb = np.where(consumed[:, None], np.float32(0.0), emb).astype(np.float32)
            alive = alive & ~consumed
        done = (not occurred) or (alive.sum() <= 1)
    return emb, alive


# revision 34
# speedup vs baseline: 1.0061x; 1.0061x over previous
"""Trainium2 kernel for nn_MergeModuleTemporal (retrieval_knn greedy merge).

Strategy:
  - Per merge round, the O(n^2 d) similarity GEMM runs on 8 NeuronCores as a
    split-bf16 3-term matmul (hi*hi + hi*lo + lo*hi) with fp32 PSUM
    accumulation -> ~1e-7 accurate vs exact fp32, which preserves every greedy
    argmax decision of the fp32 reference on this regime (decision margins are
    >= ~2.6e-7 except exact ties between bit-identical rows, which any
    deterministic matmul also ties; ties break by index).
  - The greedy scan only ever reads sim[i, j] for j > i, so each round
    computes only upper-triangular n-tile bands: m-tiles are grouped in bands
    of 8 (one per core, band t covers output rows [1024t, 1024t+1024)), and
    band t's matmuls start at column 1024t. This keeps the program identical
    across cores (SPMD) while skipping ~40% of the GEMM.
  - Rounds 1..3 compact to alive rows only (4096 -> ~2245 -> ~1273 -> ~787),
    shrinking the GEMM quadratically; per-shape programs are cached.
  - The sequential greedy matching (a tiny [n] state machine) and the exact
    fp32 fuse (clamp(a+b, max=1)) run on host between the 4 round launches;
    fp32 adds/min are bit-exact vs the jax reference.
"""

import numpy as np
import ml_dtypes
from contextlib import ExitStack

N_ROWS = 4096
D = 1024
NCORES = 8
P = 128
KO = D // P            # 8 contraction chunks of 128
MAX_STEPS = 4
THR = np.float32(0.25)
NEG = np.float32(-1e9)
NT = 512               # matmul free-dim tile
PSUM_GROUP = 4         # n-tiles sharing one k-sweep (4 tags x 2 bufs = 8 banks)
BAND = NCORES * P      # 1024 output rows per band

_PROGS = {}


def _build_program(n_pad, nbands):
    """One merge-round triangular similarity GEMM. Per core: for each band
    slot t, compute out[t*128:(t+1)*128, 1024*t:] = lhs_t.T @ rhs / D where
    lhs_t is that core's m-tile of band t (cols t*128..t*128+128 of lhs)."""
    import concourse.tile as tile
    import concourse.mybir as mybir
    from concourse import bacc

    assert n_pad % 128 == 0

    nc = bacc.Bacc(
        "TRN2",
        target_bir_lowering=False,
        debug=False,
        enable_asserts=False,
        num_devices=NCORES,
    )
    bf16 = mybir.dt.bfloat16
    f32 = mybir.dt.float32
    mcap = nbands * P  # output rows per core
    lhs_hi = nc.dram_tensor("lhs_hi", [P, KO, mcap], bf16, kind="ExternalInput").ap()
    lhs_lo = nc.dram_tensor("lhs_lo", [P, KO, mcap], bf16, kind="ExternalInput").ap()
    rhs_hi = nc.dram_tensor("rhs_hi", [P, KO, n_pad], bf16, kind="ExternalInput").ap()
    rhs_lo = nc.dram_tensor("rhs_lo", [P, KO, n_pad], bf16, kind="ExternalInput").ap()
    out = nc.dram_tensor("sim", [mcap, n_pad], f32, kind="ExternalOutput").ap()

    with tile.TileContext(nc) as tc:
        with ExitStack() as ctx:
            sb = ctx.enter_context(tc.tile_pool(name="sb", bufs=1))
            psum = ctx.enter_context(tc.tile_pool(name="psum", bufs=2, space="PSUM"))
            outp = ctx.enter_context(tc.tile_pool(name="outp", bufs=4))

            lh = sb.tile([P, KO, mcap], bf16)
            ll = sb.tile([P, KO, mcap], bf16)
            rh = sb.tile([P, KO, n_pad], bf16)
            rl = sb.tile([P, KO, n_pad], bf16)
            # Load order matches consumption order so the first matmuls can
            # start early instead of waiting for the whole operand set.
            # Granularity is shape-tuned: finer splits help big shapes but
            # fragment the DMA queue on small ones (cost-model A/B'd).
            if n_pad >= 4096:
                # Bands run high-to-low (cheapest slot first), so deliver the
                # upper column half first, k-major. k=0's upper half goes in
                # quarters (highest first) so the very first matmul only
                # waits on ~0.25MB.
                nh = n_pad // 2
                nq = (n_pad + nh) // 2
                for k in range(KO):
                    nc.sync.dma_start(lh[:, k], lhs_hi[:, k])
                    if k == 0:
                        nc.sync.dma_start(rh[:, k, nq:], rhs_hi[:, k, nq:])
                        nc.sync.dma_start(rl[:, k, nq:], rhs_lo[:, k, nq:])
                        nc.sync.dma_start(ll[:, k], lhs_lo[:, k])
                        nc.sync.dma_start(rh[:, k, nh:nq], rhs_hi[:, k, nh:nq])
                        nc.sync.dma_start(rl[:, k, nh:nq], rhs_lo[:, k, nh:nq])
                    else:
                        nc.sync.dma_start(ll[:, k], lhs_lo[:, k])
                        nc.sync.dma_start(rh[:, k, nh:], rhs_hi[:, k, nh:])
                        nc.sync.dma_start(rl[:, k, nh:], rhs_lo[:, k, nh:])
                for k in range(KO):
                    nc.sync.dma_start(rh[:, k, :nh], rhs_hi[:, k, :nh])
                    nc.sync.dma_start(rl[:, k, :nh], rhs_lo[:, k, :nh])
            elif n_pad >= 2048:
                for k in range(KO):
                    nc.sync.dma_start(lh[:, k], lhs_hi[:, k])
                    nc.sync.dma_start(ll[:, k], lhs_lo[:, k])
                    nc.sync.dma_start(rh[:, k], rhs_hi[:, k])
                    nc.sync.dma_start(rl[:, k], rhs_lo[:, k])
            else:
                nc.sync.dma_start(lh[:], lhs_hi)
                nc.sync.dma_start(ll[:], lhs_lo)
                for k in range(KO):
                    nc.sync.dma_start(rh[:, k], rhs_hi[:, k])
                    nc.sync.dma_start(rl[:, k], rhs_lo[:, k])

            reverse = n_pad >= 4096
            sched = []
            for t in (reversed(range(nbands)) if reverse else range(nbands)):
                n0 = min(t * BAND, n_pad)  # multiple of 512
                n_tiles = [(ns, min(NT, n_pad - ns)) for ns in range(n0, n_pad, NT)]
                groups = [
                    n_tiles[g0:g0 + PSUM_GROUP]
                    for g0 in range(0, len(n_tiles), PSUM_GROUP)
                ]
                for grp in (reversed(groups) if reverse else groups):
                    sched.append((t, grp))
            # Split the final group into singletons: the last stores are on
            # the critical path, and singleton groups pipeline copy+DMA per
            # tile instead of exposing the whole group's stores at the end.
            if sched and len(sched[-1][1]) > 1:
                t_last, grp_last = sched.pop()
                sched.extend((t_last, [nt]) for nt in grp_last)
            for t, grp in sched:
                ms = t * P
                pss = [
                    psum.tile([P, NT], f32, name=f"ps{j}")
                    for j in range(len(grp))
                ]
                # k-sweep over the group: weights load once per (k, term)
                # and serve len(grp) matmuls.
                for k in range(KO):
                    for ti, (lt, rt) in enumerate(((lh, rh), (lh, rl), (ll, rh))):
                        for j, (ns, nsz) in enumerate(grp):
                            nc.tensor.matmul(
                                pss[j][:, :nsz],
                                lt[:, k, ms:ms + P],
                                rt[:, k, ns:ns + nsz],
                                start=(k == 0 and ti == 0),
                                stop=(k == KO - 1 and ti == 2),
                            )
                for j, (ns, nsz) in enumerate(grp):
                    ob = outp.tile([P, NT], f32, name="ob")
                    nc.scalar.mul(ob[:, :nsz], pss[j][:, :nsz], 1.0 / D)
                    nc.sync.dma_start(out[ms:ms + P, ns:ns + nsz], ob[:, :nsz])

    nc.compile()
    return nc


def _device_sim(emb_c):
    """Upper-triangular sim = emb_c @ emb_c.T / D (fp32-accurate) on 8 cores.
    emb_c: [n_alive, D] fp32. Returns [n_alive, n_alive] fp32 with only
    j >= 1024*(i//1024-band) guaranteed valid -- in particular all j > i."""
    from concourse.bass_utils import run_bass_kernel_spmd

    n_alive = emb_c.shape[0]
    n_pad = max(256, -(-n_alive // 128) * 128)
    ntiles_m = -(-n_alive // P)
    nbands = -(-ntiles_m // NCORES)
    mcap = nbands * P

    key = (n_pad, nbands)
    if key not in _PROGS:
        _PROGS[key] = _build_program(n_pad, nbands)
    prog = _PROGS[key]

    hi = emb_c.astype(ml_dtypes.bfloat16)
    lo = (emb_c - hi.astype(np.float32)).astype(ml_dtypes.bfloat16)
    # [n, d] -> ET layout [P, KO, n_pad]  (ET[p, k, j] = emb[j, k*P + p])
    et_hi = np.zeros((P, KO, n_pad), dtype=ml_dtypes.bfloat16)
    et_lo = np.zeros((P, KO, n_pad), dtype=ml_dtypes.bfloat16)
    et_hi[:, :, :n_alive] = hi.T.reshape(KO, P, n_alive).transpose(1, 0, 2)
    et_lo[:, :, :n_alive] = lo.T.reshape(KO, P, n_alive).transpose(1, 0, 2)

    in_maps = []
    for c in range(NCORES):
        lhs_hi = np.zeros((P, KO, mcap), dtype=ml_dtypes.bfloat16)
        lhs_lo = np.zeros((P, KO, mcap), dtype=ml_dtypes.bfloat16)
        for t in range(nbands):
            mo = NCORES * t + c
            cs = mo * P
            if cs < n_pad:
                ce = min(cs + P, n_pad)
                lhs_hi[:, :, t * P:t * P + (ce - cs)] = et_hi[:, :, cs:ce]
                lhs_lo[:, :, t * P:t * P + (ce - cs)] = et_lo[:, :, cs:ce]
        in_maps.append({
            "lhs_hi": lhs_hi,
            "lhs_lo": lhs_lo,
            "rhs_hi": et_hi,
            "rhs_lo": et_lo,
        })
    res = run_bass_kernel_spmd(prog, in_maps, core_ids=list(range(NCORES)))
    sim = np.zeros((n_pad, n_pad), dtype=np.float32)
    for c in range(NCORES):
        oc = res.results[c]["sim"]  # [mcap, n_pad]
        for t in range(nbands):
            mo = NCORES * t + c
            cs = mo * P
            if cs < n_pad:
                ce = min(cs + P, n_pad)
                sim[cs:ce] = oc[t * P:t * P + (ce - cs)]
    return sim[:n_alive, :n_alive]


def _scan(sim):
    """Exact replication of the reference greedy matching on the compacted
    (all-alive) sim matrix; only reads sim[i, j>i]. argmax ties -> lowest
    index."""
    n = sim.shape[0]
    merged = np.zeros(n, dtype=bool)
    partner = np.full(n, -1)
    idx = np.arange(n)
    for i in range(n):
        cand = (idx > i) & (~merged)
        if not cand.any():
            continue
        scores = np.where(cand, sim[i], NEG)
        j = int(np.argmax(scores))
        if (not merged[i]) and (scores[j] >= THR):
            partner[i] = j
            merged[j] = True
    return partner


def kernel(embeddings):
    emb = np.asarray(embeddings, dtype=np.float32).copy()
    n = emb.shape[0]
    alive = np.ones(n, dtype=bool)
    done = False
    for _ in range(MAX_STEPS):
        if done:
            break
        alive_idx = np.nonzero(alive)[0]
        emb_c = np.ascontiguousarray(emb[alive_idx])
        sim_c = _device_sim(emb_c)
        partner_c = _scan(sim_c)
        has_c = partner_c >= 0
        occurred = bool(has_c.any())
        if occurred:
            # Map compacted indices back and replicate the reference's fp32
            # fuse exactly: fused = min(e_i + e_partner, 1); consumed -> 0.
            partner = np.full(n, -1)
            partner[alive_idx[has_c]] = alive_idx[partner_c[has_c]]
            has = partner >= 0
            consumed = np.zeros(n, dtype=bool)
            consumed[partner[has]] = True
            pidx = np.clip(partner, 0, n - 1)
            fused = np.minimum(emb + emb[pidx], np.float32(1.0))
            emb = np.where(has[:, None], fused, emb)
            emb = np.where(consumed[:, None], np.float32(0.0), emb).astype(np.float32)
            alive = alive & ~consumed
        done = (not occurred) or (alive.sum() <= 1)
    return emb, alive
